# revision 2
# baseline (speedup 1.0000x reference)
import os
import sys
from contextlib import ExitStack

import numpy as np

for _p in ("/opt/trn_rl_repo",):
    if os.path.isdir(_p) and _p not in sys.path:
        sys.path.insert(0, _p)

# Problem constants (nn_PosDecoder): out[n,l] = sum_c src[n,l,:128] . (table[1+c]*sqrt(128))
#   = src[n,l,:128] . colsum  where colsum = sqrt(128) * sum(table[1:], axis=0).
# Shard table rows across 8 cores; each core computes a partial colsum and a
# partial (N,L) output; host sums the 8 partials.
N, L, M = 16, 100, 256
F = 128
N_LOC = 100001
N_CORES = 8
R = (N_LOC - 1) // N_CORES  # 12500 table rows per core
TOK = N * L  # 1600
NBLK = R // 128  # 97 full 128-row blocks
TAIL = R - NBLK * 128  # 84
CHUNK_BLKS = 16  # 16 blocks = (128, 2048) = 1MB per DMA chunk
NCHUNK = NBLK // CHUNK_BLKS  # 6
SCALE = float(np.sqrt(F))

_BUILT = None


def _build():
    import concourse.bass as bass
    import concourse.tile as tile
    from concourse import bacc, masks, mybir

    nc = bacc.Bacc("TRN2", target_bir_lowering=False, debug=False,
                   num_devices=N_CORES)
    f32 = mybir.dt.float32
    table = nc.dram_tensor("table_slice", (R, F), f32, kind="ExternalInput").ap()
    src = nc.dram_tensor("src", (N, L, M), f32, kind="ExternalInput").ap()
    out = nc.dram_tensor("out", (1, TOK), f32, kind="ExternalOutput").ap()

    with tile.TileContext(nc) as tc, ExitStack() as ctx:
        sb = ctx.enter_context(tc.tile_pool(name="sb", bufs=1))
        chunks = ctx.enter_context(tc.tile_pool(name="chunks", bufs=3))
        srcp = ctx.enter_context(tc.tile_pool(name="srcp", bufs=3))
        pst = ctx.enter_context(
            tc.tile_pool(name="pst", bufs=2, space=bass.MemorySpace.PSUM))
        psum1 = ctx.enter_context(
            tc.tile_pool(name="psum1", bufs=1, space=bass.MemorySpace.PSUM))
        psumv = ctx.enter_context(
            tc.tile_pool(name="psumv", bufs=2, space=bass.MemorySpace.PSUM))

        identity = sb.tile([128, 128], f32)
        masks.make_identity(nc, identity[:])
        ones = sb.tile([128, 1], f32)
        nc.gpsimd.memset(ones[:], SCALE)  # fold the sqrt(F) scale into the reduce
        srcT = sb.tile([128, 13 * 128], f32)
        out_sb = sb.tile([1, TOK], f32)
        acc = sb.tile([128, CHUNK_BLKS * F], f32)

        # src (tokens, feat) -> srcT (feat, tokens) via PE transposes; runs
        # early, hidden under the table stream.
        src_flat = src.rearrange("n l m -> (n l) m")  # (1600, 256)
        ntile = (TOK + 127) // 128  # 13
        for ti in range(ntile):
            t0 = ti * 128
            tw = min(128, TOK - t0)
            s = srcp.tile([128, F], f32)
            if tw < 128:
                nc.gpsimd.memset(s[:], 0.0)
            nc.sync.dma_start(s[:tw, :], src_flat[t0:t0 + tw, 0:F])
            pt = pst.tile([128, 128], f32)
            nc.tensor.transpose(pt[:], s[:], identity[:])
            nc.vector.tensor_copy(srcT[:, t0:t0 + 128], pt[:])

        # Table stream. Row r lands on partition r//NBLK-ish via the
        # "(p t) f" split so each partition reads one long contiguous run;
        # row placement is irrelevant since we sum everything.
        main = table[0:NBLK * 128, :].rearrange("(p t) f -> p (t f)", p=128)
        first = None
        for c in range(NCHUNK):
            ch = chunks.tile([128, CHUNK_BLKS * F], f32)
            nc.sync.dma_start(
                ch[:], main[:, c * CHUNK_BLKS * F:(c + 1) * CHUNK_BLKS * F])
            if c == 0:
                first = ch
            elif c == 1:
                nc.vector.tensor_add(acc[:], first[:], ch[:])
            else:
                nc.vector.tensor_add(acc[:], acc[:], ch[:])
        b96 = sb.tile([128, F], f32)
        nc.sync.dma_start(b96[:], main[:, NCHUNK * CHUNK_BLKS * F:])
        tailt = sb.tile([TAIL, F], f32)
        nc.sync.dma_start(tailt[:], table[NBLK * 128:R, :])

        # Partition-reduce everything into one PSUM column:
        # colsum[f] = sum_p block[p, f] across all blocks (PSUM accumulation).
        cps = psum1.tile([128, 1], f32)
        for i in range(CHUNK_BLKS):
            nc.tensor.matmul(cps[:], acc[:, i * F:(i + 1) * F], ones[:],
                             start=(i == 0), stop=False)
        nc.tensor.matmul(cps[:], b96[:], ones[:], start=False, stop=False)
        nc.tensor.matmul(cps[:], tailt[:], ones[:TAIL, :], start=False, stop=True)

        colsum = sb.tile([128, 1], f32)
        nc.vector.tensor_copy(colsum[:], cps[:])

        # out_row = colsum^T @ srcT  -> (1, 1600)
        for j in range(0, TOK, 512):
            w = min(512, TOK - j)
            pv = psumv.tile([1, 512], f32)
            nc.tensor.matmul(pv[:1, :w], colsum[:], srcT[:, j:j + w],
                             start=True, stop=True)
            nc.vector.tensor_copy(out_sb[:, j:j + w], pv[:1, :w])
        nc.sync.dma_start(out[:], out_sb[:])

    nc.compile()
    return nc


def kernel(src=None, ds=None, lookup_table=None, **_):
    global _BUILT
    if _BUILT is None:
        _BUILT = _build()
    nc = _BUILT
    from concourse import bass_utils

    src = np.ascontiguousarray(np.asarray(src, dtype=np.float32))
    tab = np.asarray(lookup_table, dtype=np.float32)
    in_maps = []
    for k in range(N_CORES):
        sl = np.ascontiguousarray(tab[1 + k * R:1 + (k + 1) * R, :])
        in_maps.append({"table_slice": sl, "src": src})
    res = bass_utils.run_bass_kernel_spmd(nc, in_maps,
                                          core_ids=list(range(N_CORES)))
    parts = [next(iter(r.values())).reshape(-1) for r in res.results]
    total = np.sum(np.stack(parts, 0), axis=0, dtype=np.float64)
    return total.astype(np.float32).reshape(N, L)


# revision 4
# speedup vs baseline: 1.1565x; 1.1565x over previous
import os
import sys
from contextlib import ExitStack

import numpy as np

for _p in ("/opt/trn_rl_repo",):
    if os.path.isdir(_p) and _p not in sys.path:
        sys.path.insert(0, _p)

# Problem (nn_PosDecoder): out[n,l] = sum_c src[n,l,:128] . (table[1+c]*sqrt(128))
#   = src[n,l,:128] . colsum  where colsum = sqrt(128) * sum(table[1:], axis=0).
# Shard table rows across 8 cores; each core computes a partial colsum and a
# partial (N,L) output row; host sums the 8 partial rows.
N, L, M = 16, 100, 256
F = 128
N_LOC = 100001
N_CORES = 8
R = (N_LOC - 1) // N_CORES  # 12500 table rows per core
TOK = N * L  # 1600
NBLK = R // 128  # 97 full 128-row blocks
TAIL = R - NBLK * 128  # 84
SCALE = float(np.sqrt(F))

# Table blocks split across the two HWDGE queues (sync + act).
SQ_W = [16, 16, 8, 4, 2, 2, 1]  # 49 blocks, DVE tensor_reduce per chunk
AQ_W = 8                        # act queue: 6 chunks of 8 blocks -> POOL adds
AQ_N = 6                        # 48 blocks
ACCW = AQ_W * F  # 1024

_BUILT = None


def _build():
    import concourse.bass as bass
    import concourse.tile as tile
    from concourse import bacc, mybir

    assert sum(SQ_W) + AQ_W * AQ_N == NBLK
    nc = bacc.Bacc("TRN2", target_bir_lowering=False, debug=False,
                   num_devices=N_CORES)
    f32 = mybir.dt.float32
    table = nc.dram_tensor("table_slice", (R, F), f32, kind="ExternalInput").ap()
    srcT = nc.dram_tensor("srcT", (F, TOK), f32, kind="ExternalInput").ap()
    out = nc.dram_tensor("out", (1, TOK), f32, kind="ExternalOutput").ap()

    with tile.TileContext(nc) as tc, ExitStack() as ctx:
        sb = ctx.enter_context(tc.tile_pool(name="sb", bufs=1))
        schunks = ctx.enter_context(tc.tile_pool(name="schunks", bufs=3))
        achunks = ctx.enter_context(tc.tile_pool(name="achunks", bufs=3))
        parts = ctx.enter_context(tc.tile_pool(name="parts", bufs=3))
        psum1 = ctx.enter_context(
            tc.tile_pool(name="psum1", bufs=1, space=bass.MemorySpace.PSUM))
        psumv = ctx.enter_context(
            tc.tile_pool(name="psumv", bufs=2, space=bass.MemorySpace.PSUM))

        ones = sb.tile([128, 1], f32)
        nc.gpsimd.memset(ones[:], SCALE)  # folds the sqrt(F) scale into colsum
        srcT_sb = sb.tile([128, TOK], f32)
        out_sb = sb.tile([1, TOK], f32)
        acc_b = sb.tile([128, ACCW], f32)
        tailt = sb.tile([TAIL, F], f32)

        main = table[0:NBLK * 128, :].rearrange("(p t) f -> p (t f)", p=128)

        # --- sync HWDGE queue: SQ chunks (descending sizes)
        scol = [0]
        for w in SQ_W:
            scol.append(scol[-1] + w * F)
        s_tiles = []
        for ci, w in enumerate(SQ_W):
            ch = schunks.tile([128, w * F], f32)
            nc.sync.dma_start(ch[:], main[:, scol[ci]:scol[ci + 1]])
            s_tiles.append(ch)

        # --- act HWDGE queue: tail rows, AQ chunks, then srcT
        abase = scol[-1]
        nc.scalar.dma_start(tailt[:], table[NBLK * 128:R, :])
        a_tiles = []
        nc.scalar.dma_start(acc_b[:], main[:, abase:abase + ACCW])
        for ci in range(1, AQ_N):
            ch = achunks.tile([128, ACCW], f32)
            nc.scalar.dma_start(
                ch[:], main[:, abase + ci * ACCW:abase + (ci + 1) * ACCW])
            a_tiles.append(ch)
        nc.scalar.dma_start(srcT_sb[:], srcT[:, :])

        # --- DVE: fold each sync chunk (p, blk*F) -> (p, F) in one reduce.
        s_parts = []
        for ci, w in enumerate(SQ_W):
            if w == 1:
                s_parts.append(s_tiles[ci])
                continue
            pr = parts.tile([128, F], f32)
            nc.vector.tensor_reduce(
                pr[:], s_tiles[ci].rearrange("p (b f) -> p f b", f=F),
                axis=mybir.AxisListType.X, op=mybir.AluOpType.add)
            s_parts.append(pr)

        # --- POOL: accumulate act chunks into acc_b, then DVE-fold it.
        for ch in a_tiles:
            nc.gpsimd.tensor_add(acc_b[:], acc_b[:], ch[:])
        part_b = parts.tile([128, F], f32)
        nc.vector.tensor_reduce(
            part_b[:], acc_b.rearrange("p (b f) -> p f b", f=F),
            axis=mybir.AxisListType.X, op=mybir.AluOpType.add)

        # --- PE: one PSUM accumulation group -> colsum (128,1)
        cps = psum1.tile([128, 1], f32)
        nc.tensor.matmul(cps[:], tailt[:], ones[:TAIL, :], start=True,
                         stop=False)
        for pr in s_parts:
            nc.tensor.matmul(cps[:], pr[:], ones[:], start=False, stop=False)
        nc.tensor.matmul(cps[:], part_b[:], ones[:], start=False, stop=True)
        colsum = sb.tile([128, 1], f32)
        nc.vector.tensor_copy(colsum[:], cps[:])

        # --- out_row = colsum^T @ srcT -> (1, 1600)
        for i, j in enumerate(range(0, TOK, 512)):
            w = min(512, TOK - j)
            pv = psumv.tile([1, 512], f32)
            nc.tensor.matmul(pv[:1, :w], colsum[:], srcT_sb[:, j:j + w],
                             start=True, stop=True)
            if i % 2 == 0:
                nc.vector.tensor_copy(out_sb[:, j:j + w], pv[:1, :w])
            else:
                nc.scalar.copy(out_sb[:, j:j + w], pv[:1, :w])
        nc.sync.dma_start(out[:], out_sb[:])

    nc.compile()
    return nc


def make_in_maps(src, lookup_table):
    src_f = np.asarray(src, dtype=np.float32).reshape(TOK, M)[:, :F]
    srcT_np = np.ascontiguousarray(src_f.T)  # (128, 1600)
    tab = np.asarray(lookup_table, dtype=np.float32)
    in_maps = []
    for k in range(N_CORES):
        sl = np.ascontiguousarray(tab[1 + k * R:1 + (k + 1) * R, :])
        in_maps.append({"table_slice": sl, "srcT": srcT_np})
    return in_maps


def kernel(src=None, ds=None, lookup_table=None, **_):
    global _BUILT
    if _BUILT is None:
        _BUILT = _build()
    from concourse import bass_utils

    in_maps = make_in_maps(src, lookup_table)
    res = bass_utils.run_bass_kernel_spmd(_BUILT, in_maps,
                                          core_ids=list(range(N_CORES)))
    parts = [next(iter(r.values())).reshape(-1) for r in res.results]
    total = np.sum(np.stack(parts, 0), axis=0, dtype=np.float64)
    return total.astype(np.float32).reshape(N, L)
